# revision 1
# baseline (speedup 1.0000x reference)
"""Trainium2 Bass kernel for the KPC fusion module (dense_transformer).

Strategy (sequence-parallel over the point dimension N):
  - 8 cores, each takes NS = N/8 = 4096 points for both batch elements.
  - The curves preprocessing (attention over curves, p_l/p_n, k/v projections)
    is tiny and replicated on every core.
  - Per-point work is a fused two-softmax attention. Scores are built in
    transposed layout S^T[l, n] so both matmuls need no transposes:
        S^T = K2^T @ x        (K2 = (Wc/sqrt(MID))^T Wb ga folds q away)
        P   = exp(S^T)        (no max-subtraction: |S| <= ~3)
        [Num; Den] = [v|1]^T @ P   (PSUM accumulation over l-tiles)
    The tiny "inter" attention (3 keys) rides the same machinery as a 17th
    l-tile whose v/ones columns live in rows 4..7 of the accumulator.
  - LayerNorm/Conv1d/BatchNorm epilogue runs on a (b, channel, subchunk)
    packed [96, 512] layout so every DVE/ACT op uses ~96-128 partitions.
  - BatchNorm batch stats: one AllReduce of 12 floats across the 8 cores.

All matmul inputs are bitcast to float32r (TF32-like, 1 cycle/row) — fp32
matmul is 4x slower on TRN2. Verified end-to-end precision ~2.5e-3 relative
to the fp32 reference (threshold-safe).
"""

import numpy as np

B = 2
C = 6
N = 32768
MID = 3
NCV = 3          # curve count (NC dim)
L = 2048
NCORES = 8
NS = N // NCORES          # 4096 points per core per batch
EPS = 1e-5
LT = 17                   # 16 intra l-tiles of 128 + 1 inter tile
NCH = NS // 512           # 8 n-chunks of 512 per batch
SCH = 8                   # subchunks of 512 for packed epilogue
LP = 4                    # l-chunks of 512
LEXT = LT * 128           # 2176

_cache = {}
DEBUG_DUMP = False


def _host_consts(inputs):
    """Pure weight algebra + constant routing matrices (host-side prep)."""
    f32 = np.float32
    Wa, Wav, Wb, Wbv, Wc, Wd = (np.asarray(inputs[k], f32) for k in
                                ["Wa", "Wav", "Wb", "Wbv", "Wc", "Wd"])
    Watt = np.asarray(inputs["Watt"], f32)
    ln_g, ln_b = np.asarray(inputs["ln_gamma"], f32), np.asarray(inputs["ln_beta"], f32)
    Wpl, bpl = np.asarray(inputs["Wpl"], f32), np.asarray(inputs["bpl"], f32)
    Wpn, bpn = np.asarray(inputs["Wpn"], f32), np.asarray(inputs["bpn"], f32)

    scale = np.sqrt(f32(MID))
    Wc_s = (Wc / scale).astype(f32)           # [3,6]
    WA = (Wc_s.T @ Wb).astype(f32)            # [6,6] K2_intra = WA @ ga
    WB = (Wc_s.T @ Wa).astype(f32)            # [6,6] K2_inter = WB @ gi
    wbar = Wpn.mean(axis=0).astype(f32)       # [3]
    bbar = float(bpn.mean())
    W1 = (Wd * ln_g[None, :]).astype(f32)     # [6,6]
    c0 = (Wd @ ln_b).astype(f32)              # [6]

    consts = {}
    # att[s,:] = sum_c Watt[c] * curves_pk[(s,c),:] as one block-diag matmul
    Watt_map = np.zeros((72, 12), f32)
    for s in range(12):
        for c in range(C):
            Watt_map[s * 6 + c, s] = Watt[c]
    consts["Watt_map"] = Watt_map

    # --- preprocessing maps.  curves_pk partition order: p = s*6 + c,
    # s = n*LP + lp (12 chunks of 512 covering the flattened (n,l) axis).
    map_l = np.zeros((12, 3), f32)            # s -> n   (sum over l of a row)
    map_l2 = np.zeros((3, 12), f32)           # n -> s   (replicate)
    map_n = np.zeros((12, 4), f32)            # s -> lp  (sum over n)
    map_n2 = np.zeros((4, 12), f32)           # lp -> s  (replicate)
    for s in range(12):
        n, lp = divmod(s, LP)
        map_l[s, n] = 1.0
        map_l2[n, s] = 1.0
        map_n[s, lp] = 1.0
        map_n2[lp, s] = 1.0
    consts["map_l"] = map_l
    consts["map_l2"] = map_l2
    consts["map_n"] = map_n
    consts["map_n2"] = map_n2

    rep_c = np.zeros((12, 72), f32)           # s -> (s,c)
    for s in range(12):
        for c in range(C):
            rep_c[s, s * 6 + c] = 1.0
    consts["rep_c"] = rep_c

    # reductions from (s=(n,lp), c) packed [72] rows:
    map_red_ci = np.zeros((72, 18), f32)      # (s,c) -> (c,n)  [sum over lp]
    map_pl = np.zeros((72, 18), f32)          # (s=(k,lp),c) -> (c,m): Wpl[m,k]/L
    map_ci = np.zeros((72, 24), f32)          # (s=(n,lp),c) -> (c,lp) [sum over n]
    map_pn = np.zeros((72, 24), f32)          # (s=(k,lp),c) -> (c,lp): wbar[k]
    for s in range(12):
        n, lp = divmod(s, LP)
        for c in range(C):
            p = s * 6 + c
            map_red_ci[p, c * 3 + n] = 1.0
            for m in range(MID):
                map_pl[p, c * 3 + m] = Wpl[m, n] / L
            map_ci[p, c * LP + lp] = 1.0
            map_pn[p, c * LP + lp] = wbar[n]
    consts["map_red_ci"] = map_red_ci
    consts["map_pl"] = map_pl
    consts["map_ci"] = map_ci
    consts["map_pn"] = map_pn

    consts["bpl_rep"] = np.tile(bpl, C).reshape(18, 1)   # (c,m) -> bpl[m]

    WA_map = np.zeros((24, 24), f32)          # ga (c,lp) -> K2 (c',lp)
    for lp in range(LP):
        for c in range(C):
            for c2 in range(C):
                WA_map[c * LP + lp, c2 * LP + lp] = WA[c2, c]
    consts["WA_map"] = WA_map
    consts["WB_T"] = WB.T.copy()              # [6,6] lhsT for K2i = WB @ gi
    consts["WavT"] = Wav.T.copy()             # [6,3]
    consts["WbvT"] = Wbv.T.copy()             # [6,3]

    # --- epilogue maps on (b, c, s) packed [96] rows
    map_mean = np.zeros((96, 16), f32)
    rep16 = np.zeros((16, 96), f32)
    W1_map = np.zeros((96, 96), f32)
    c0_rep = np.zeros((96, 1), f32)
    bn_map = np.zeros((96, 6), f32)
    rep_ad = np.zeros((6, 96), f32)
    for b in range(B):
        for c in range(C):
            for s in range(SCH):
                p = b * 48 + c * SCH + s
                map_mean[p, b * SCH + s] = 1.0 / C
                rep16[b * SCH + s, p] = 1.0
                for o in range(C):
                    W1_map[p, b * 48 + o * SCH + s] = W1[o, c]
                c0_rep[p, 0] = c0[c]
                bn_map[p, c] = 1.0
                rep_ad[c, p] = 1.0
    consts["map_mean"] = map_mean
    consts["rep16"] = rep16
    consts["W1_map"] = W1_map
    consts["c0_rep"] = c0_rep
    consts["bn_map"] = bn_map
    consts["rep_ad"] = rep_ad

    consts["bn_gamma_c"] = np.asarray(inputs["bn_gamma"], f32).reshape(6, 1)
    consts["bn_beta_c"] = np.asarray(inputs["bn_beta"], f32).reshape(6, 1)
    return consts, bbar


def _build(const_shapes, bbar):
    import concourse.bacc as bacc
    import concourse.mybir as mybir
    import concourse.tile as tile

    dt = mybir.dt
    f32 = dt.float32
    f32r = dt.float32r
    AF = mybir.ActivationFunctionType
    ALU = mybir.AluOpType
    AX = mybir.AxisListType

    nc = bacc.Bacc(
        "TRN2", target_bir_lowering=False, debug=False, num_devices=NCORES
    )

    x_d = nc.dram_tensor("x_sh", [B, C, NS], f32, kind="ExternalInput")
    curves_d = nc.dram_tensor("curves", [B, C, NCV, L], f32, kind="ExternalInput")
    cd = {}
    for name, shp in const_shapes.items():
        cd[name] = nc.dram_tensor(name, list(shp), f32, kind="ExternalInput")
    out_d = nc.dram_tensor("out", [B, C, NS], f32, kind="ExternalOutput")
    dbg = {}
    if DEBUG_DUMP:
        dbg["nd"] = nc.dram_tensor("dbg_nd", [B, 8, NCH, 512], f32, kind="ExternalOutput")
        dbg["ga"] = nc.dram_tensor("dbg_ga", [B, 24, 512], f32, kind="ExternalOutput")
        dbg["gi"] = nc.dram_tensor("dbg_gi", [B, 18, 1], f32, kind="ExternalOutput")
        dbg["k2e"] = nc.dram_tensor("dbg_k2e", [B, C, LEXT], f32, kind="ExternalOutput")
        dbg["ve"] = nc.dram_tensor("dbg_ve", [B, 128, LT, 8], f32, kind="ExternalOutput")
        dbg["cfp"] = nc.dram_tensor("dbg_cfp", [96, 512], f32, kind="ExternalOutput")
        dbg["cfn"] = nc.dram_tensor("dbg_cfn", [96, 512], f32, kind="ExternalOutput")
        dbg["ysb"] = nc.dram_tensor("dbg_ysb", [96, 512], f32, kind="ExternalOutput")

    with tile.TileContext(nc) as tc:
        with (
            tc.tile_pool(name="const", bufs=1) as constp,
            tc.tile_pool(name="pre", bufs=1) as pre,
            tc.tile_pool(name="prepsum", bufs=2, space="PSUM") as pps,
            tc.tile_pool(name="spsum", bufs=2, space="PSUM") as spool,
            tc.tile_pool(name="ndpsum", bufs=2, space="PSUM") as ndpool,
            tc.tile_pool(name="ptile", bufs=3) as ppool,
            tc.tile_pool(name="epi", bufs=1) as epi,
            tc.tile_pool(name="epipsum", bufs=2, space="PSUM") as eps_,
            tc.tile_pool(name="dram", bufs=1, space="DRAM") as dram,
        ):
            # ---- load constants ----
            cs = {}
            for name, shp in const_shapes.items():
                t = constp.tile(list(shp), f32, tag=name)
                nc.sync.dma_start(t[:], cd[name][:])
                cs[name] = t

            # ---- load x (channel-major per batch, and packed for residual) ----
            x_sb = []
            for b in range(B):
                t = pre.tile([C, NS], f32r, tag=f"x{b}")
                nc.sync.dma_start(t[:], x_d[b].bitcast(f32r))
                x_sb.append(t)
            xp_sb = epi.tile([96, 512], f32, tag="xp")
            nc.sync.dma_start(
                xp_sb[:], x_d[:].rearrange("b c (s j) -> (b c s) j", j=512)
            )

            # ---- preprocessing: curves -> K2ext, vext per batch ----
            K2ext, vext = [], []
            red_p0 = None  # [72,1] free-sums of batch-0 curves (for p_l / p_n)
            cpk_all = []
            for b in range(B):
                cpk = pre.tile([72, 512], f32, tag=f"cpk{b}")
                nc.sync.dma_start(
                    cpk[:],
                    curves_d[b].rearrange("c n (lp j) -> c (n lp) j", j=512)
                    .transpose([1, 0, 2]),
                )
                cpk_all.append(cpk)

            for b in range(B):
                cpk = cpk_all[b]
                # att[n,l] scores packed as (s=(n,lp)) rows
                att_ps = pps.tile([12, 512], f32, tag="ps")
                nc.tensor.matmul(att_ps[:], (cs["Watt_map"][:]), (cpk[:]),
                                 start=True, stop=True)
                E_att = pre.tile([12, 512], f32, tag="eatt")
                nc.scalar.activation(E_att[:], att_ps[:], AF.Exp)

                sums_s = pre.tile([12, 1], f32, tag="sums_s")
                nc.vector.reduce_sum(sums_s[:], E_att[:], axis=AX.X)

                # softmax over l (per curve row n)
                dl_ps = pps.tile([3, 1], f32, tag="ps")
                nc.tensor.matmul(dl_ps[:], (cs["map_l"][:]), (sums_s[:]),
                                 start=True, stop=True)
                rl = pre.tile([3, 1], f32, tag="rl")
                nc.vector.reciprocal(rl[:], dl_ps[:])
                rl_rep_ps = pps.tile([12, 1], f32, tag="ps")
                nc.tensor.matmul(rl_rep_ps[:], (cs["map_l2"][:]), (rl[:]),
                                 start=True, stop=True)
                rl_rep = pre.tile([12, 1], f32, tag="rlrepsb")
                nc.vector.tensor_copy(rl_rep[:], rl_rep_ps[:])
                sm_l = pre.tile([12, 512], f32, tag="sml")
                nc.vector.tensor_scalar_mul(sm_l[:], E_att[:], rl_rep[:])

                # softmax over n (3 rows per l)
                dn_ps = pps.tile([4, 512], f32, tag="ps")
                nc.tensor.matmul(dn_ps[:], (cs["map_n"][:]), (E_att[:]),
                                 start=True, stop=True)
                rn = pre.tile([4, 512], f32, tag="rn")
                nc.vector.reciprocal(rn[:], dn_ps[:])
                rn_rep_ps = pps.tile([12, 512], f32, tag="ps")
                nc.tensor.matmul(rn_rep_ps[:], (cs["map_n2"][:]), (rn[:]),
                                 start=True, stop=True)
                sm_n = pre.tile([12, 512], f32, tag="smn")
                nc.vector.tensor_tensor(sm_n[:], E_att[:], rn_rep_ps[:], ALU.mult)

                # weighted curve sums
                sml_rep_ps = pps.tile([72, 512], f32, tag="ps")
                nc.tensor.matmul(sml_rep_ps[:], (cs["rep_c"][:]), (sm_l[:]),
                                 start=True, stop=True)
                prod_i = pre.tile([72, 512], f32, tag="prodi")
                nc.vector.tensor_tensor(prod_i[:], cpk[:], sml_rep_ps[:], ALU.mult)
                smn_rep_ps = pps.tile([72, 512], f32, tag="ps")
                nc.tensor.matmul(smn_rep_ps[:], (cs["rep_c"][:]), (sm_n[:]),
                                 start=True, stop=True)
                prod_n = pre.tile([72, 512], f32, tag="prodn")
                nc.vector.tensor_tensor(prod_n[:], cpk[:], smn_rep_ps[:], ALU.mult)

                red_i = pre.tile([72, 1], f32, tag="redi")
                nc.vector.reduce_sum(red_i[:], prod_i[:], axis=AX.X)
                if b == 0:
                    red_p0 = pre.tile([72, 1], f32, tag="redp0")
                    nc.vector.reduce_sum(red_p0[:], cpk[:], axis=AX.X)

                # gi = curver_inter + p_l   [18,1] packed (c,m)
                gi_ps = pps.tile([18, 1], f32, tag="ps")
                nc.tensor.matmul(gi_ps[:], (cs["map_red_ci"][:]), (red_i[:]),
                                 start=True, stop=False)
                nc.tensor.matmul(gi_ps[:], (cs["map_pl"][:]), (red_p0[:]),
                                 start=False, stop=True)
                gi18 = pre.tile([18, 1], f32, tag="gi18")
                nc.vector.tensor_scalar_add(gi18[:], gi_ps[:], cs["bpl_rep"][:])
                gi_c = pre.tile([C, 3], f32, tag=f"gic{b}")
                nc.sync.dma_start(gi_c[:], gi18[:])
                if DEBUG_DUMP:
                    nc.sync.dma_start(dbg["gi"][b], gi18[:])

                # ga = curves_intra + p_n   [24,512] packed (c,lp)
                ga_ps = pps.tile([24, 512], f32, tag="ps")
                nc.tensor.matmul(ga_ps[:], (cs["map_ci"][:]), (prod_n[:]),
                                 start=True, stop=False)
                nc.tensor.matmul(ga_ps[:], (cs["map_pn"][:]), (cpk_all[0][:]),
                                 start=False, stop=True)
                ga_sb = pre.tile([24, 512], f32, tag=f"ga{b}")
                nc.scalar.activation(ga_sb[:], ga_ps[:], AF.Copy, bias=float(bbar))
                ga_c6 = pre.tile([C, L], f32, tag=f"gac6{b}")
                nc.sync.dma_start(ga_c6[:], ga_sb[:])
                if DEBUG_DUMP:
                    nc.sync.dma_start(dbg["ga"][b], ga_sb[:])

                # K2ext [6, 2176]
                k2_ps = pps.tile([24, 512], f32, tag="ps")
                nc.tensor.matmul(k2_ps[:], (cs["WA_map"][:]), (ga_sb[:]),
                                 start=True, stop=True)
                k2_24 = pre.tile([24, 512], f32, tag="k224")
                nc.vector.tensor_copy(k2_24[:], k2_ps[:])
                k2e = pre.tile([C, LEXT], f32r, tag=f"k2e{b}")
                nc.vector.memset(k2e[:, 2048:].bitcast(f32), 0.0)
                nc.sync.dma_start(k2e[:, 0:2048], k2_24[:].bitcast(f32r))
                k2i_ps = pps.tile([C, 3], f32, tag="ps")
                nc.tensor.matmul(k2i_ps[:], (cs["WB_T"][:]), (gi_c[:]),
                                 start=True, stop=True)
                nc.vector.tensor_copy(k2e[:, 2048:2051], k2i_ps[:])
                if DEBUG_DUMP:
                    nc.sync.dma_start(dbg["k2e"][b], k2e[:].bitcast(f32))
                K2ext.append(k2e)

                # vext [128, 17, 8]
                ve = pre.tile([128, LT, 8], f32r, tag=f"ve{b}")
                nc.vector.memset(ve[:].bitcast(f32), 0.0)
                nc.vector.memset(ve[:, 0:16, 3:4].bitcast(f32), 1.0)
                for t in range(16):
                    v_ps = pps.tile([128, 3], f32, tag="ps")
                    nc.tensor.matmul(
                        v_ps[:], (ga_c6[:, 128 * t : 128 * (t + 1)]),
                        (cs["WbvT"][:]), start=True, stop=True,
                    )
                    nc.vector.tensor_copy(ve[:, t, 0:3], v_ps[:])
                vi_ps = pps.tile([3, 3], f32, tag="ps")
                nc.tensor.matmul(vi_ps[:], (gi_c[:]), (cs["WavT"][:]),
                                 start=True, stop=True)
                nc.vector.tensor_copy(ve[0:3, 16, 4:7], vi_ps[:])
                nc.vector.memset(ve[0:3, 16, 7:8].bitcast(f32), 1.0)
                if DEBUG_DUMP:
                    nc.sync.dma_start(dbg["ve"][b], ve[:].bitcast(f32))
                vext.append(ve)

            # ---- main loop: fused double attention ----
            nd_sb = []
            for b in range(B):
                t = epi.tile([8, NCH, 512], f32, tag=f"nd{b}")
                nd_sb.append(t)
            for b in range(B):
                for ch in range(NCH):
                    nd_ps = ndpool.tile([8, 512], f32)
                    xr = (x_sb[b][:, 512 * ch : 512 * (ch + 1)])
                    for t in range(LT):
                        s_ps = spool.tile([128, 512], f32)
                        nc.tensor.matmul(
                            s_ps[:],
                            (K2ext[b][:, 128 * t : 128 * (t + 1)]),
                            xr,
                            start=True, stop=True,
                        )
                        p_t = ppool.tile([128, 512], f32r)
                        nc.scalar.activation(p_t[:], s_ps[:], AF.Exp)
                        nc.tensor.matmul(
                            nd_ps[:], (vext[b][:, t, :]), (p_t[:]),
                            start=(t == 0), stop=(t == LT - 1),
                        )
                    nc.vector.tensor_copy(nd_sb[b][:, ch, :], nd_ps[:])

            if DEBUG_DUMP:
                for b2 in range(B):
                    nc.sync.dma_start(dbg["nd"][b2], nd_sb[b2][:])

            # ---- epilogue on packed [96, 512] (b, c, s) ----
            cf_p = epi.tile([96, 512], f32, tag="cfp")
            den_p = epi.tile([96, 512], f32, tag="denp")
            for b in range(B):
                # cf rows: c 0-2 <- inter Num (nd rows 4-6); c 3-5 <- intra Num (0-2)
                nc.sync.dma_start(cf_p[b * 48 : b * 48 + 24, :], nd_sb[b][4:7])
                nc.sync.dma_start(cf_p[b * 48 + 24 : b * 48 + 48, :], nd_sb[b][0:3])
                for c in range(3):
                    nc.sync.dma_start(den_p[b * 48 + c * 8 : b * 48 + c * 8 + 8, :],
                                      nd_sb[b][7:8])
                    nc.sync.dma_start(
                        den_p[b * 48 + 24 + c * 8 : b * 48 + 32 + c * 8, :],
                        nd_sb[b][3:4])
            if DEBUG_DUMP:
                nc.sync.dma_start(dbg["cfp"][:], cf_p[:])
            binv = epi.tile([96, 512], f32, tag="binv")
            nc.vector.reciprocal(binv[:], den_p[:])
            cfn = epi.tile([96, 512], f32, tag="cfn")
            nc.vector.tensor_tensor(cfn[:], cf_p[:], binv[:], ALU.mult)
            if DEBUG_DUMP:
                nc.sync.dma_start(dbg["cfn"][:], cfn[:])
            cf2 = epi.tile([96, 512], f32, tag="cf2")
            nc.vector.tensor_tensor(cf2[:], cfn[:], cfn[:], ALU.mult)

            mu_ps = eps_.tile([16, 512], f32, tag="eps")
            nc.tensor.matmul(mu_ps[:], (cs["map_mean"][:]), (cfn[:]),
                             start=True, stop=True)
            m2_ps = eps_.tile([16, 512], f32, tag="eps")
            nc.tensor.matmul(m2_ps[:], (cs["map_mean"][:]), (cf2[:]),
                             start=True, stop=True)
            mu_sb = epi.tile([16, 512], f32, tag="mu")
            nc.vector.tensor_copy(mu_sb[:], mu_ps[:])
            musq = epi.tile([16, 512], f32, tag="musq")
            nc.vector.tensor_tensor(musq[:], mu_sb[:], mu_sb[:], ALU.mult)
            var_sb = epi.tile([16, 512], f32, tag="var")
            nc.vector.tensor_tensor(var_sb[:], m2_ps[:], musq[:], ALU.subtract)
            eps16 = epi.tile([16, 1], f32, tag="eps16")
            nc.vector.memset(eps16[:], EPS)
            sv = epi.tile([16, 512], f32, tag="sv")
            nc.scalar.activation(sv[:], var_sb[:], AF.Sqrt, bias=eps16[:])
            r_sb = epi.tile([16, 512], f32, tag="rsb")
            nc.vector.reciprocal(r_sb[:], sv[:])
            mur = epi.tile([16, 512], f32, tag="mur")
            nc.vector.tensor_tensor(mur[:], mu_sb[:], r_sb[:], ALU.mult)

            rrep_ps = eps_.tile([96, 512], f32, tag="eps")
            nc.tensor.matmul(rrep_ps[:], (cs["rep16"][:]), (r_sb[:]),
                             start=True, stop=True)
            murrep_ps = eps_.tile([96, 512], f32, tag="eps")
            nc.tensor.matmul(murrep_ps[:], (cs["rep16"][:]), (mur[:]),
                             start=True, stop=True)
            z1 = epi.tile([96, 512], f32, tag="z1")
            nc.vector.tensor_tensor(z1[:], cfn[:], rrep_ps[:], ALU.mult)
            z = epi.tile([96, 512], f32, tag="z")
            nc.vector.tensor_tensor(z[:], z1[:], murrep_ps[:], ALU.subtract)

            y_ps = eps_.tile([96, 512], f32, tag="eps")
            nc.tensor.matmul(y_ps[:], (cs["W1_map"][:]), (z[:]),
                             start=True, stop=True)
            y_sb = epi.tile([96, 512], f32, tag="ysb")
            nc.vector.tensor_scalar_add(y_sb[:], y_ps[:], cs["c0_rep"][:])
            if DEBUG_DUMP:
                nc.sync.dma_start(dbg["ysb"][:], y_sb[:])
            y2 = epi.tile([96, 512], f32, tag="y2")
            nc.vector.tensor_tensor(y2[:], y_sb[:], y_sb[:], ALU.mult)

            bsum_ps = eps_.tile([6, 512], f32, tag="eps")
            nc.tensor.matmul(bsum_ps[:], (cs["bn_map"][:]), (y_sb[:]),
                             start=True, stop=True)
            bsq_ps = eps_.tile([6, 512], f32, tag="eps")
            nc.tensor.matmul(bsq_ps[:], (cs["bn_map"][:]), (y2[:]),
                             start=True, stop=True)
            partials = epi.tile([6, 2], f32, tag="partials")
            nc.vector.reduce_sum(partials[:, 0:1], bsum_ps[:], axis=AX.X)
            nc.vector.reduce_sum(partials[:, 1:2], bsq_ps[:], axis=AX.X)

            # ---- AllReduce of the 12 BN partial sums ----
            cc_in = dram.tile([6, 2], f32, tag="ccin")
            cc_out = dram.tile([6, 2], f32, tag="ccout")
            nc.gpsimd.dma_start(cc_in[:], partials[:])
            nc.gpsimd.collective_compute(
                "AllReduce", mybir.AluOpType.add,
                replica_groups=[list(range(NCORES))],
                ins=[cc_in[:].opt()], outs=[cc_out[:].opt()],
            )
            stats_g = epi.tile([6, 2], f32, tag="statsg")
            nc.gpsimd.dma_start(stats_g[:], cc_out[:])

            # ---- final BN scalars ----
            inv_cnt = 1.0 / float(B * N)
            bm = epi.tile([6, 1], f32, tag="bm")
            nc.vector.tensor_scalar_mul(bm[:], stats_g[:, 0:1], inv_cnt)
            m2g = epi.tile([6, 1], f32, tag="m2g")
            nc.vector.tensor_scalar_mul(m2g[:], stats_g[:, 1:2], inv_cnt)
            bmsq = epi.tile([6, 1], f32, tag="bmsq")
            nc.vector.tensor_tensor(bmsq[:], bm[:], bm[:], ALU.mult)
            bv = epi.tile([6, 1], f32, tag="bv")
            nc.vector.tensor_tensor(bv[:], m2g[:], bmsq[:], ALU.subtract)
            svb = epi.tile([6, 1], f32, tag="svb")
            nc.scalar.activation(svb[:], bv[:], AF.Sqrt, bias=eps16[0:6, :])
            rb = epi.tile([6, 1], f32, tag="rb")
            nc.vector.reciprocal(rb[:], svb[:])
            a6 = epi.tile([6, 1], f32, tag="a6")
            nc.vector.tensor_tensor(a6[:], rb[:], cs["bn_gamma_c"][:], ALU.mult)
            t1 = epi.tile([6, 1], f32, tag="t1")
            nc.vector.tensor_tensor(t1[:], a6[:], bm[:], ALU.mult)
            d6 = epi.tile([6, 1], f32, tag="d6")
            nc.vector.tensor_tensor(d6[:], cs["bn_beta_c"][:], t1[:], ALU.subtract)

            ad_ps = eps_.tile([96, 2], f32, tag="eps")
            nc.tensor.matmul(ad_ps[:, 0:1], (cs["rep_ad"][:]), (a6[:]),
                             start=True, stop=True)
            nc.tensor.matmul(ad_ps[:, 1:2], (cs["rep_ad"][:]), (d6[:]),
                             start=True, stop=True)
            ad_sb = epi.tile([96, 2], f32, tag="adsb")
            nc.vector.tensor_copy(ad_sb[:], ad_ps[:])

            # ---- BN apply + residual + LeakyReLU + store ----
            t5 = epi.tile([96, 512], f32, tag="t5")
            nc.vector.tensor_scalar(t5[:], y_sb[:], ad_sb[:, 0:1], ad_sb[:, 1:2],
                                    ALU.mult, ALU.add)
            t6 = epi.tile([96, 512], f32, tag="t6")
            nc.vector.tensor_tensor(t6[:], t5[:], xp_sb[:], ALU.add)
            t7 = epi.tile([96, 512], f32, tag="t7")
            nc.vector.tensor_scalar_mul(t7[:], t6[:], 0.2)
            outp = epi.tile([96, 512], f32, tag="outp")
            nc.vector.tensor_tensor(outp[:], t6[:], t7[:], ALU.max)
            nc.sync.dma_start(
                out_d[:].rearrange("b c (s j) -> (b c s) j", j=512), outp[:]
            )

    nc.compile()
    return nc


def kernel(**inputs):
    from concourse.bass_utils import run_bass_kernel_spmd

    consts, bbar = _host_consts(inputs)
    const_shapes = {k: v.shape for k, v in consts.items()}

    key = ("v1", bbar) + tuple(sorted((k, v) for k, v in const_shapes.items()))
    if key not in _cache:
        _cache[key] = _build(const_shapes, bbar)
    nc = _cache[key]

    x = np.ascontiguousarray(np.asarray(inputs["x"], np.float32))
    curves = np.ascontiguousarray(np.asarray(inputs["curves"], np.float32))

    in_maps = []
    for i in range(NCORES):
        m = {"curves": curves}
        m.update(consts)
        m["x_sh"] = np.ascontiguousarray(x[:, :, i * NS : (i + 1) * NS])
        in_maps.append(m)

    res = run_bass_kernel_spmd(nc, in_maps, core_ids=list(range(NCORES)))
    out = np.empty((B, C, N), np.float32)
    for i in range(NCORES):
        out[:, :, i * NS : (i + 1) * NS] = res.results[i]["out"]
    if DEBUG_DUMP:
        return out, res.results
    return out



# revision 14
# speedup vs baseline: 1.3170x; 1.3170x over previous
"""Trainium2 Bass kernel for the KPC fusion module (dense_transformer).

Sequence-parallel over N (8 cores x 4096 points x 2 batches). Per core the
fused double-softmax attention runs as a software-pipelined stream over 17
l-tiles x 16 point-chunks:

    S^T = K2^T @ x            (PE, f32r; q folded into K2)
    P   = exp(S^T)            (odd tiles: ACT exact Exp;
                               even tiles: DVE Schraudolph fast-exp, one
                               tensor_scalar into int32 bitcast as float)
    [Num;Den] += v^T @ P      (PE PSUM accumulation)

Emission is interleaved (scores run 3 tiles ahead of the accumulates) so the
PE never waits on exp and stays at full p-state; exp work is split between
ACT and DVE so neither falls behind the PE (~7.3us per chunk).

Epilogue: LayerNorm is scale-invariant per point, so instead of dividing by
softmax denominators we multiply each feature group by the *other* group's
denominator (no reciprocal). BN batch stats: one 48-byte AllReduce.
"""

import numpy as np

B = 2
C = 6
N = 32768
MID = 3
NCV = 3
L = 2048
NCORES = 8
NS = N // NCORES
EPS = 1e-5
LT = 17                   # 16 intra l-tiles of 128 + 1 inter tile
NCH = NS // 512           # 8 point chunks of 512 per batch
SCH = 8
LP = 4
LEXT = LT * 128

LA = 3                    # accum matmuls trail score matmuls by LA tiles
# bf16-domain Schraudolph fast-exp: exp(x) ~= bitcast_bf16(int16(A*x + B))
A_EXP = float(2 ** 7 / np.log(2))
B_EXP = float(127 * 2 ** 7 - 366393.0 / 65536.0)

_cache = {}


def _host_consts(inputs):
    """Weight algebra + constant routing matrices, packed into one block."""
    f32 = np.float32
    Wa, Wav, Wb, Wbv, Wc, Wd = (np.asarray(inputs[k], f32) for k in
                                ["Wa", "Wav", "Wb", "Wbv", "Wc", "Wd"])
    Watt = np.asarray(inputs["Watt"], f32)
    ln_g, ln_b = np.asarray(inputs["ln_gamma"], f32), np.asarray(inputs["ln_beta"], f32)
    Wpl, bpl = np.asarray(inputs["Wpl"], f32), np.asarray(inputs["bpl"], f32)
    Wpn, bpn = np.asarray(inputs["Wpn"], f32), np.asarray(inputs["bpn"], f32)

    scale = np.sqrt(f32(MID))
    Wc_s = (Wc / scale).astype(f32)
    WA = (Wc_s.T @ Wb).astype(f32)            # [6,6] K2_intra = WA @ ga
    WB = (Wc_s.T @ Wa).astype(f32)            # [6,6] K2_inter = WB @ gi
    wbar = Wpn.mean(axis=0).astype(f32)
    bbar = float(bpn.mean())
    W1 = (Wd * ln_g[None, :]).astype(f32)
    c0 = (Wd @ ln_b).astype(f32)

    consts = {}
    Watt_map = np.zeros((72, 12), f32)
    for s in range(12):
        for c in range(C):
            Watt_map[s * 6 + c, s] = Watt[c]
    consts["Watt_map"] = Watt_map

    map_l = np.zeros((12, 3), f32)
    map_l2 = np.zeros((3, 12), f32)
    map_n = np.zeros((12, 4), f32)
    map_n2 = np.zeros((4, 12), f32)
    for s in range(12):
        n, lp = divmod(s, LP)
        map_l[s, n] = 1.0
        map_l2[n, s] = 1.0
        map_n[s, lp] = 1.0
        map_n2[lp, s] = 1.0
    consts["map_l"] = map_l
    consts["map_l2"] = map_l2
    consts["map_n"] = map_n
    consts["map_n2"] = map_n2

    rep_c = np.zeros((12, 72), f32)
    for s in range(12):
        for c in range(C):
            rep_c[s, s * 6 + c] = 1.0
    consts["rep_c"] = rep_c

    map_red_ci = np.zeros((72, 18), f32)
    map_pl = np.zeros((72, 18), f32)
    map_ci = np.zeros((72, 24), f32)
    map_pn = np.zeros((72, 24), f32)
    for s in range(12):
        n, lp = divmod(s, LP)
        for c in range(C):
            p = s * 6 + c
            map_red_ci[p, c * 3 + n] = 1.0
            for m in range(MID):
                map_pl[p, c * 3 + m] = Wpl[m, n] / L
            map_ci[p, c * LP + lp] = 1.0
            map_pn[p, c * LP + lp] = wbar[n]
    consts["map_red_ci"] = map_red_ci
    consts["map_pl"] = map_pl
    consts["map_ci"] = map_ci
    consts["map_pn"] = map_pn

    consts["bpl_rep"] = np.tile(bpl, C).reshape(18, 1)

    WA_map = np.zeros((24, 24), f32)
    for lp in range(LP):
        for c in range(C):
            for c2 in range(C):
                WA_map[c * LP + lp, c2 * LP + lp] = WA[c2, c]
    consts["WA_map"] = WA_map
    consts["WB_T"] = WB.T.copy()
    consts["WavT"] = Wav.T.copy()
    consts["WbvT"] = Wbv.T.copy()

    # epilogue maps on (b, c, s) packed [96] rows
    map_mean = np.zeros((96, 16), f32)
    rep16 = np.zeros((16, 96), f32)
    W1_map = np.zeros((96, 96), f32)
    c0_rep = np.zeros((96, 1), f32)
    bn_map = np.zeros((96, 6), f32)
    rep_ad = np.zeros((6, 96), f32)
    for b in range(B):
        for c in range(C):
            for s in range(SCH):
                p = b * 48 + c * SCH + s
                map_mean[p, b * SCH + s] = 1.0 / C
                rep16[b * SCH + s, p] = 1.0
                for o in range(C):
                    W1_map[p, b * 48 + o * SCH + s] = W1[o, c]
                c0_rep[p, 0] = c0[c]
                bn_map[p, c] = 1.0
                rep_ad[c, p] = 1.0
    consts["map_mean"] = map_mean
    consts["rep16"] = rep16
    consts["W1_map"] = W1_map
    consts["c0_rep"] = c0_rep
    consts["bn_map"] = bn_map
    consts["rep_ad"] = rep_ad

    consts["bn_gamma_c"] = np.asarray(inputs["bn_gamma"], f32).reshape(6, 1)
    consts["bn_beta_c"] = np.asarray(inputs["bn_beta"], f32).reshape(6, 1)

    # pack everything into one [128, F] block (one DMA on device)
    layout = {}
    off = 0
    for k, v in consts.items():
        r, ccols = v.shape
        layout[k] = (r, off, ccols)
        off += ccols
    pack = np.zeros((128, off), f32)
    for k, v in consts.items():
        r, o, ccols = layout[k]
        pack[:r, o:o + ccols] = v
    return pack, layout, bbar


def _build(pack_shape, layout, bbar):
    import concourse.bacc as bacc
    import concourse.mybir as mybir
    import concourse.tile as tile

    dt = mybir.dt
    f32 = dt.float32
    f32r = dt.float32r
    i16 = dt.int16
    bf16 = dt.bfloat16
    AF = mybir.ActivationFunctionType
    ALU = mybir.AluOpType
    AX = mybir.AxisListType

    nc = bacc.Bacc(
        "TRN2", target_bir_lowering=False, debug=False, num_devices=NCORES
    )

    x_d = nc.dram_tensor("x_sh", [B, C, NS], f32, kind="ExternalInput")
    curves_d = nc.dram_tensor("curves", [B, C, NCV, L], f32, kind="ExternalInput")
    pack_d = nc.dram_tensor("cpack", list(pack_shape), f32, kind="ExternalInput")
    out_d = nc.dram_tensor("out", [B, C, NS], f32, kind="ExternalOutput")

    with tile.TileContext(nc) as tc:
        with (
            tc.tile_pool(name="const", bufs=1) as constp,
            tc.tile_pool(name="pre", bufs=1) as pre,
            tc.tile_pool(name="aux", bufs=2, space="PSUM") as aux,
            tc.tile_pool(name="spsum", bufs=4, space="PSUM") as spool,
            tc.tile_pool(name="ndpsum", bufs=2, space="PSUM") as ndpool,
            tc.tile_pool(name="pact", bufs=2) as pact,
            tc.tile_pool(name="pdve", bufs=2) as pdve,
            tc.tile_pool(name="epi", bufs=1) as epi,
            tc.tile_pool(name="dram", bufs=1, space="DRAM") as dram,
        ):
            # ---- input loads (main-loop feeds first) ----
            x_sb = []
            for b in range(B):
                t = pre.tile([C, NS], f32r, tag=f"x{b}")
                nc.sync.dma_start(t[:], x_d[b].bitcast(f32r))
                x_sb.append(t)
            cpk_all = []
            for b in range(B):
                cpk = pre.tile([72, 512], f32, tag=f"cpk{b}")
                nc.sync.dma_start(
                    cpk[:],
                    curves_d[b].rearrange("c n (lp j) -> c (n lp) j", j=512)
                    .transpose([1, 0, 2]),
                )
                cpk_all.append(cpk)
            pk = constp.tile(list(pack_shape), f32, tag="cpack")
            nc.sync.dma_start(pk[:], pack_d[:])
            cs = {k: pk[0:r, o:o + w] for k, (r, o, w) in layout.items()}
            xp_sb = epi.tile([96, 512], f32, tag="xp")
            nc.sync.dma_start(
                xp_sb[:], x_d[:].rearrange("b c (s j) -> (b c s) j", j=512)
            )

            # ---- CC warm-up: dummy AllReduce well before the real one ----
            cc_in = dram.tile([6, 2], f32, tag="ccin")
            cc_out = dram.tile([6, 2], f32, tag="ccout")
            warm = epi.tile([6, 2], f32, tag="warm")
            nc.vector.memset(warm[:], 0.0)
            nc.gpsimd.dma_start(cc_in[:], warm[:])
            nc.gpsimd.collective_compute(
                "AllReduce", mybir.AluOpType.add,
                replica_groups=[list(range(NCORES))],
                ins=[cc_in[:].opt()], outs=[cc_out[:].opt()],
            )

            # ---- preprocessing, stage-major over both batches ----
            E_att, sums_s, sm_l, sm_n = [None, None], [None, None], [None, None], [None, None]
            prod_i, prod_n, red_i = [None, None], [None, None], [None, None]
            gi_c, ga_sb, ga_c6 = [None, None], [None, None], [None, None]
            K2ext, vext, vext_bf = [None, None], [None, None], [None, None]
            red_p0 = None

            for b in range(B):
                att_ps = aux.tile([12, 512], f32, tag="ps")
                nc.tensor.matmul(att_ps[:], cs["Watt_map"], cpk_all[b][:],
                                 start=True, stop=True)
                E_att[b] = pre.tile([12, 512], f32, name=f"eatt{b}", tag=f"eatt{b}")
                nc.scalar.activation(E_att[b][:], att_ps[:], AF.Exp)
            for b in range(B):
                sums_s[b] = pre.tile([12, 1], f32, name=f"sums{b}", tag=f"sums{b}")
                nc.vector.reduce_sum(sums_s[b][:], E_att[b][:], axis=AX.X)
            for b in range(B):
                dl_ps = aux.tile([3, 1], f32, tag="ps")
                nc.tensor.matmul(dl_ps[:], cs["map_l"], sums_s[b][:],
                                 start=True, stop=True)
                rl = pre.tile([3, 1], f32, tag=f"rl{b}")
                nc.vector.reciprocal(rl[:], dl_ps[:])
                rl_rep_ps = aux.tile([12, 1], f32, tag="ps")
                nc.tensor.matmul(rl_rep_ps[:], cs["map_l2"], rl[:],
                                 start=True, stop=True)
                rl_rep = pre.tile([12, 1], f32, tag=f"rlrep{b}")
                nc.vector.tensor_copy(rl_rep[:], rl_rep_ps[:])
                sm_l[b] = pre.tile([12, 512], f32, name=f"sml{b}", tag=f"sml{b}")
                nc.vector.tensor_scalar_mul(sm_l[b][:], E_att[b][:], rl_rep[:])
            for b in range(B):
                dn_ps = aux.tile([4, 512], f32, tag="ps")
                nc.tensor.matmul(dn_ps[:], cs["map_n"], E_att[b][:],
                                 start=True, stop=True)
                rn = pre.tile([4, 512], f32, tag=f"rn{b}")
                nc.vector.reciprocal(rn[:], dn_ps[:])
                rn_rep_ps = aux.tile([12, 512], f32, tag="ps")
                nc.tensor.matmul(rn_rep_ps[:], cs["map_n2"], rn[:],
                                 start=True, stop=True)
                sm_n[b] = pre.tile([12, 512], f32, name=f"smn{b}", tag=f"smn{b}")
                nc.vector.tensor_tensor(sm_n[b][:], E_att[b][:], rn_rep_ps[:],
                                        ALU.mult)
            for b in range(B):
                sml_rep_ps = aux.tile([72, 512], f32, tag="ps")
                nc.tensor.matmul(sml_rep_ps[:], cs["rep_c"], sm_l[b][:],
                                 start=True, stop=True)
                prod_i[b] = pre.tile([72, 512], f32, name=f"prodi{b}", tag=f"prodi{b}")
                nc.vector.tensor_tensor(prod_i[b][:], cpk_all[b][:],
                                        sml_rep_ps[:], ALU.mult)
                smn_rep_ps = aux.tile([72, 512], f32, tag="ps")
                nc.tensor.matmul(smn_rep_ps[:], cs["rep_c"], sm_n[b][:],
                                 start=True, stop=True)
                prod_n[b] = pre.tile([72, 512], f32, name=f"prodn{b}", tag=f"prodn{b}")
                nc.vector.tensor_tensor(prod_n[b][:], cpk_all[b][:],
                                        smn_rep_ps[:], ALU.mult)
            red_p0 = pre.tile([72, 1], f32, tag="redp0")
            nc.vector.reduce_sum(red_p0[:], cpk_all[0][:], axis=AX.X)
            for b in range(B):
                red_i[b] = pre.tile([72, 1], f32, name=f"redi{b}", tag=f"redi{b}")
                nc.vector.reduce_sum(red_i[b][:], prod_i[b][:], axis=AX.X)
            for b in range(B):
                gi_ps = aux.tile([18, 1], f32, tag="ps")
                nc.tensor.matmul(gi_ps[:], cs["map_red_ci"], red_i[b][:],
                                 start=True, stop=False)
                nc.tensor.matmul(gi_ps[:], cs["map_pl"], red_p0[:],
                                 start=False, stop=True)
                gi18 = pre.tile([18, 1], f32, tag=f"gi18{b}")
                nc.vector.tensor_scalar_add(gi18[:], gi_ps[:], cs["bpl_rep"])
                gi_c[b] = pre.tile([C, 3], f32, name=f"gic{b}", tag=f"gic{b}")
                nc.sync.dma_start(gi_c[b][:], gi18[:])
            for b in range(B):
                ga_ps = aux.tile([24, 512], f32, tag="ps")
                nc.tensor.matmul(ga_ps[:], cs["map_ci"], prod_n[b][:],
                                 start=True, stop=False)
                nc.tensor.matmul(ga_ps[:], cs["map_pn"], cpk_all[0][:],
                                 start=False, stop=True)
                ga_sb[b] = pre.tile([24, 512], f32, name=f"ga{b}", tag=f"ga{b}")
                nc.scalar.activation(ga_sb[b][:], ga_ps[:], AF.Copy,
                                     bias=float(bbar))
                ga_c6[b] = pre.tile([C, L], f32, name=f"gac6{b}", tag=f"gac6{b}")
                nc.sync.dma_start(ga_c6[b][:], ga_sb[b][:])
            for b in range(B):
                k2_ps = aux.tile([24, 512], f32, tag="ps")
                nc.tensor.matmul(k2_ps[:], cs["WA_map"], ga_sb[b][:],
                                 start=True, stop=True)
                k2_24 = pre.tile([24, 512], f32, tag=f"k224{b}")
                nc.vector.tensor_copy(k2_24[:], k2_ps[:])
                k2e = pre.tile([C, LEXT], f32r, tag=f"k2e{b}")
                nc.vector.memset(k2e[:, 2048:].bitcast(f32), 0.0)
                nc.sync.dma_start(k2e[:, 0:2048], k2_24[:].bitcast(f32r))
                k2i_ps = aux.tile([C, 3], f32, tag="ps")
                nc.tensor.matmul(k2i_ps[:], cs["WB_T"], gi_c[b][:],
                                 start=True, stop=True)
                nc.vector.tensor_copy(k2e[:, 2048:2051], k2i_ps[:])
                K2ext[b] = k2e
            for b in range(B):
                # all 17 v-tiles into one PSUM tile, one copy out
                vps = aux.tile([128, LT, 8], f32, tag="ps")
                nc.vector.memset(vps[:], 0.0)
                for t in range(16):
                    nc.tensor.matmul(
                        vps[:, t, 0:3], ga_c6[b][:, 128 * t: 128 * (t + 1)],
                        cs["WbvT"], start=True, stop=True,
                    )
                nc.tensor.matmul(vps[0:3, 16, 4:7], gi_c[b][:], cs["WavT"],
                                 start=True, stop=True)
                nc.vector.memset(vps[:, 0:16, 3:4], 1.0)
                nc.vector.memset(vps[0:3, 16, 7:8], 1.0)
                ve = pre.tile([128, LT, 8], f32r, tag=f"ve{b}")
                nc.vector.tensor_copy(ve[:], vps[:])
                vext[b] = ve
                veb = pre.tile([128, LT, 8], bf16, name=f"veb{b}",
                               tag=f"veb{b}")
                nc.vector.tensor_copy(veb[:], vps[:])
                vext_bf[b] = veb

            # ---- main loop: software-pipelined fused double attention ----
            cf_p = epi.tile([96, 512], f32, tag="cfp")      # numerators
            mult_p = epi.tile([96, 512], f32, tag="multp")  # opposite denoms
            nd_sb = []
            for b in range(B):
                nd_sb.append(epi.tile([8, NCH, 512], f32, name=f"nd{b}", tag=f"nd{b}"))

            total = B * NCH * LT
            s_tiles = [None] * (LA + 1)     # rotating score-psum refs
            p_tiles = [None] * (LA + 1)     # rotating exp output refs
            nd_ps = None

            def chunk_of(k):
                return divmod(k // LT, NCH)  # -> (b, ch)

            def emit_score(k):
                b, ch = chunk_of(k)
                t = k % LT
                s_ps = spool.tile([128, 512], f32)
                nc.tensor.matmul(
                    s_ps[:], K2ext[b][:, 128 * t: 128 * (t + 1)],
                    x_sb[b][:, 512 * ch: 512 * (ch + 1)],
                    start=True, stop=True,
                )
                s_tiles[k % (LA + 1)] = s_ps

            def emit_exp(k):
                t = k % LT
                s_ps = s_tiles[k % (LA + 1)]
                if t % 2 == 1:
                    p_t = pact.tile([128, 512], f32r)
                    nc.scalar.activation(p_t[:], s_ps[:], AF.Exp)
                else:
                    p_t = pdve.tile([128, 512], i16)
                    nc.vector.tensor_scalar(p_t[:], s_ps[:], A_EXP, B_EXP,
                                            ALU.mult, ALU.add)
                p_tiles[k % (LA + 1)] = p_t

            def emit_accum(k):
                nonlocal nd_ps
                b, ch = chunk_of(k)
                t = k % LT
                if t == 0:
                    nd_ps = ndpool.tile([8, 512], f32)
                p_t = p_tiles[k % (LA + 1)]
                if t % 2 == 1:
                    nc.tensor.matmul(
                        nd_ps[:], vext[b][:, t, :], p_t[:],
                        start=(t == 0), stop=(t == LT - 1),
                    )
                else:
                    nc.tensor.matmul(
                        nd_ps[:], vext_bf[b][:, t, :], p_t[:].bitcast(bf16),
                        start=(t == 0), stop=(t == LT - 1),
                    )
                if t == LT - 1:
                    # copy out + feed the epilogue repack (DMAs hide under
                    # the main loop)
                    nd = nd_sb[b]
                    nc.vector.tensor_copy(nd[:, ch, :], nd_ps[:])
                    r0 = b * 48 + ch
                    nc.sync.dma_start(
                        cf_p[r0:r0 + 17:8, :], nd[4:7, ch, :])
                    nc.sync.dma_start(
                        cf_p[r0 + 24:r0 + 41:8, :], nd[0:3, ch, :])
                    for c in range(3):
                        nc.sync.dma_start(
                            mult_p[r0 + 8 * c:r0 + 8 * c + 1, :],
                            nd[3:4, ch, :])
                        nc.sync.dma_start(
                            mult_p[r0 + 24 + 8 * c:r0 + 25 + 8 * c, :],
                            nd[7:8, ch, :])

            for k in range(total):
                emit_score(k)
                emit_exp(k)
                if k >= LA:
                    emit_accum(k - LA)
            for k in range(total - LA, total):
                emit_accum(k)

            # ---- epilogue on packed [96, 512] (b, c, s) rows ----
            cfn = epi.tile([96, 512], f32, tag="cfn")
            nc.vector.tensor_tensor(cfn[:], cf_p[:], mult_p[:], ALU.mult)
            cf2 = epi.tile([96, 512], f32, tag="cf2")
            nc.vector.tensor_tensor(cf2[:], cfn[:], cfn[:], ALU.mult)

            mu_ps = aux.tile([16, 512], f32, tag="ps")
            nc.tensor.matmul(mu_ps[:], cs["map_mean"], cfn[:],
                             start=True, stop=True)
            m2_ps = aux.tile([16, 512], f32, tag="ps")
            nc.tensor.matmul(m2_ps[:], cs["map_mean"], cf2[:],
                             start=True, stop=True)
            mu_sb = epi.tile([16, 512], f32, tag="mu")
            nc.vector.tensor_copy(mu_sb[:], mu_ps[:])
            musq = epi.tile([16, 512], f32, tag="musq")
            nc.vector.tensor_tensor(musq[:], mu_sb[:], mu_sb[:], ALU.mult)
            var_sb = epi.tile([16, 512], f32, tag="var")
            nc.vector.tensor_tensor(var_sb[:], m2_ps[:], musq[:], ALU.subtract)
            eps16 = epi.tile([16, 1], f32, tag="eps16")
            nc.vector.memset(eps16[:], EPS)
            sv = epi.tile([16, 512], f32, tag="sv")
            nc.scalar.activation(sv[:], var_sb[:], AF.Sqrt, bias=eps16[:])
            r_sb = epi.tile([16, 512], f32, tag="rsb")
            nc.vector.reciprocal(r_sb[:], sv[:])
            mur = epi.tile([16, 512], f32, tag="mur")
            nc.vector.tensor_tensor(mur[:], mu_sb[:], r_sb[:], ALU.mult)

            rrep_ps = aux.tile([96, 512], f32, tag="ps")
            nc.tensor.matmul(rrep_ps[:], cs["rep16"], r_sb[:],
                             start=True, stop=True)
            murrep_ps = aux.tile([96, 512], f32, tag="ps")
            nc.tensor.matmul(murrep_ps[:], cs["rep16"], mur[:],
                             start=True, stop=True)
            z1 = epi.tile([96, 512], f32, tag="z1")
            nc.vector.tensor_tensor(z1[:], cfn[:], rrep_ps[:], ALU.mult)
            z = epi.tile([96, 512], f32, tag="z")
            nc.vector.tensor_tensor(z[:], z1[:], murrep_ps[:], ALU.subtract)

            y_ps = aux.tile([96, 512], f32, tag="ps")
            nc.tensor.matmul(y_ps[:], cs["W1_map"], z[:],
                             start=True, stop=True)
            y_sb = epi.tile([96, 512], f32, tag="ysb")
            nc.vector.tensor_scalar_add(y_sb[:], y_ps[:], cs["c0_rep"])
            y2 = epi.tile([96, 512], f32, tag="y2")
            nc.vector.tensor_tensor(y2[:], y_sb[:], y_sb[:], ALU.mult)

            bsum_ps = aux.tile([6, 512], f32, tag="ps")
            nc.tensor.matmul(bsum_ps[:], cs["bn_map"], y_sb[:],
                             start=True, stop=True)
            bsq_ps = aux.tile([6, 512], f32, tag="ps")
            nc.tensor.matmul(bsq_ps[:], cs["bn_map"], y2[:],
                             start=True, stop=True)
            partials = epi.tile([6, 2], f32, tag="partials")
            nc.vector.reduce_sum(partials[:, 0:1], bsum_ps[:], axis=AX.X)
            nc.vector.reduce_sum(partials[:, 1:2], bsq_ps[:], axis=AX.X)

            # ---- AllReduce of the 12 BN partial sums ----
            nc.gpsimd.dma_start(cc_in[:], partials[:])
            nc.gpsimd.collective_compute(
                "AllReduce", mybir.AluOpType.add,
                replica_groups=[list(range(NCORES))],
                ins=[cc_in[:].opt()], outs=[cc_out[:].opt()],
            )
            stats_g = epi.tile([6, 2], f32, tag="statsg")
            nc.gpsimd.dma_start(stats_g[:], cc_out[:])

            # ---- final BN scalars ----
            inv_cnt = 1.0 / float(B * N)
            bm = epi.tile([6, 1], f32, tag="bm")
            nc.vector.tensor_scalar_mul(bm[:], stats_g[:, 0:1], inv_cnt)
            m2g = epi.tile([6, 1], f32, tag="m2g")
            nc.vector.tensor_scalar_mul(m2g[:], stats_g[:, 1:2], inv_cnt)
            bmsq = epi.tile([6, 1], f32, tag="bmsq")
            nc.vector.tensor_tensor(bmsq[:], bm[:], bm[:], ALU.mult)
            bv = epi.tile([6, 1], f32, tag="bv")
            nc.vector.tensor_tensor(bv[:], m2g[:], bmsq[:], ALU.subtract)
            svb = epi.tile([6, 1], f32, tag="svb")
            nc.scalar.activation(svb[:], bv[:], AF.Sqrt, bias=eps16[0:6, :])
            rb = epi.tile([6, 1], f32, tag="rb")
            nc.vector.reciprocal(rb[:], svb[:])
            a6 = epi.tile([6, 1], f32, tag="a6")
            nc.vector.tensor_tensor(a6[:], rb[:], cs["bn_gamma_c"], ALU.mult)
            t1 = epi.tile([6, 1], f32, tag="t1")
            nc.vector.tensor_tensor(t1[:], a6[:], bm[:], ALU.mult)
            d6 = epi.tile([6, 1], f32, tag="d6")
            nc.vector.tensor_tensor(d6[:], cs["bn_beta_c"], t1[:], ALU.subtract)

            ad_ps = aux.tile([96, 2], f32, tag="ps")
            nc.tensor.matmul(ad_ps[:, 0:1], cs["rep_ad"], a6[:],
                             start=True, stop=True)
            nc.tensor.matmul(ad_ps[:, 1:2], cs["rep_ad"], d6[:],
                             start=True, stop=True)
            ad_sb = epi.tile([96, 2], f32, tag="adsb")
            nc.vector.tensor_copy(ad_sb[:], ad_ps[:])

            # ---- BN apply + residual + LeakyReLU + store ----
            t5 = epi.tile([96, 512], f32, tag="t5")
            nc.vector.tensor_scalar(t5[:], y_sb[:], ad_sb[:, 0:1],
                                    ad_sb[:, 1:2], ALU.mult, ALU.add)
            t6 = epi.tile([96, 512], f32, tag="t6")
            nc.vector.tensor_tensor(t6[:], t5[:], xp_sb[:], ALU.add)
            t7 = epi.tile([96, 512], f32, tag="t7")
            nc.vector.tensor_scalar_mul(t7[:], t6[:], 0.2)
            outp = epi.tile([96, 512], f32, tag="outp")
            nc.vector.tensor_tensor(outp[:], t6[:], t7[:], ALU.max)
            nc.sync.dma_start(
                out_d[:].rearrange("b c (s j) -> (b c s) j", j=512), outp[:]
            )

    nc.compile()
    return nc


def kernel(**inputs):
    from concourse.bass_utils import run_bass_kernel_spmd

    pack, layout, bbar = _host_consts(inputs)

    key = ("v2", bbar, pack.shape, tuple(sorted(layout.items())))
    if key not in _cache:
        _cache[key] = _build(pack.shape, layout, bbar)
    nc = _cache[key]

    x = np.ascontiguousarray(np.asarray(inputs["x"], np.float32))
    curves = np.ascontiguousarray(np.asarray(inputs["curves"], np.float32))

    in_maps = []
    for i in range(NCORES):
        in_maps.append({
            "curves": curves,
            "cpack": pack,
            "x_sh": np.ascontiguousarray(x[:, :, i * NS: (i + 1) * NS]),
        })

    res = run_bass_kernel_spmd(nc, in_maps, core_ids=list(range(NCORES)))
    out = np.empty((B, C, N), np.float32)
    for i in range(NCORES):
        out[:, :, i * NS: (i + 1) * NS] = res.results[i]["out"]
    return out
